# revision 9
# baseline (speedup 1.0000x reference)
"""CrossModalMatchingNetwork Trainium2 kernel.

Full-input contract: kernel(**inputs) takes the unsharded numpy inputs and
returns the full [B, S, S] cosine-similarity output (float32).

Strategy: data-parallel over batch across 8 NeuronCores (2 batches/core).
Host-side prep transposes the big activations to [D, S] layout so the
contraction dim lands on SBUF partitions, casts them to bf16 (fp32 PSUM
accumulation keeps the error ~3e-3), and replicates the small projection
weights (pre-transposed to [D, H]) to every core.

Per core, per batch:
  vT[h,s]  = sum_d WvT[d,h] * visT[d,s] + bv[h]     (bf16 matmuls, fp32 acc)
  tT[h,s]  = sum_d WtT[d,h] * txtT[d,s] + bt[h]
  vn2[s]   = sum_h vT[h,s]^2   (DVE squares + ones-vector matmul)
  tn2[s]   = sum_h tT[h,s]^2
  vT      *= 1/sqrt(vn2)       (sqrt row -> replicate via K=1 matmul -> DVE
                                reciprocal on all 128 lanes -> fold into vT)
  dots     = vT.T @ tT
  out      = dots * 1/sqrt(tn2)  (DVE epilogue, fp32 out)
"""

import numpy as np
from contextlib import ExitStack

import concourse.bass as bass
import concourse.mybir as mybir
import concourse.tile as tile
from concourse import bacc
from concourse.bass import ds, ts

B, S, VD, TD, H = 16, 1024, 1024, 768, 512
NCORES = 8
BPC = B // NCORES  # batches per core
P = 128
FD = 512  # matmul moving-operand free dim (one PSUM bank of fp32)

F32 = mybir.dt.float32
F32R = mybir.dt.float32r
BF16 = mybir.dt.bfloat16

AF = mybir.ActivationFunctionType


def build(bpc=BPC, s=S, vd=VD, td=TD, h=H, dtype="bf16"):
    fd = min(FD, s)
    fdm = fd  # one PSUM bank per matmul: moving free dim capped at 512
    kv, kt, mh = vd // P, td // P, h // P
    ns, ms = s // fd, s // P
    nsm = s // fdm

    if dtype == "bf16":
        CT = BF16
        _w = lambda ap: ap  # noqa: E731
    else:
        CT = F32
        _w = lambda ap: ap.bitcast(F32R)  # noqa: E731  # fp32r: single-pass PE

    nc = bacc.Bacc("TRN2", target_bir_lowering=False)
    visT = nc.dram_tensor("visT", [bpc, vd, s], CT, kind="ExternalInput")
    txtT = nc.dram_tensor("txtT", [bpc, td, s], CT, kind="ExternalInput")
    wvT = nc.dram_tensor("wvT", [vd, h], CT, kind="ExternalInput")
    wtT = nc.dram_tensor("wtT", [td, h], CT, kind="ExternalInput")
    bvp = nc.dram_tensor("bvp", [P, mh], F32, kind="ExternalInput")
    btp = nc.dram_tensor("btp", [P, mh], F32, kind="ExternalInput")
    onesd = nc.dram_tensor("ones", [P, P], CT, kind="ExternalInput")
    out = nc.dram_tensor("out", [bpc, s, s], F32, kind="ExternalOutput")

    with (
        tile.TileContext(nc) as tc,
        ExitStack() as ctx,
        nc.allow_low_precision(reason="compute dtype is bf16/fp32r by design"),
    ):
        consts = ctx.enter_context(tc.tile_pool(name="consts", bufs=1))
        vis_pool = ctx.enter_context(tc.tile_pool(name="vis", bufs=1))
        txt_pool = ctx.enter_context(tc.tile_pool(name="txt", bufs=1))
        vt_pool = ctx.enter_context(tc.tile_pool(name="vt", bufs=1))
        tt_pool = ctx.enter_context(tc.tile_pool(name="tt", bufs=1))
        sq_pool = ctx.enter_context(tc.tile_pool(name="sq", bufs=1))
        row_pool = ctx.enter_context(tc.tile_pool(name="rows", bufs=2))
        rvn_pool = ctx.enter_context(tc.tile_pool(name="rvn", bufs=2))
        rtn_pool = ctx.enter_context(tc.tile_pool(name="rtn", bufs=2))
        out_pool = ctx.enter_context(tc.tile_pool(name="outs", bufs=3))
        ps_mm = ctx.enter_context(tc.tile_pool(name="ps_mm", bufs=4 * FD // fdm, space="PSUM"))
        ps_repl = ctx.enter_context(tc.tile_pool(name="ps_repl", bufs=2, space="PSUM"))
        ps_norm = ctx.enter_context(tc.tile_pool(name="ps_norm", bufs=2, space="PSUM"))

        # --- constants: weights (as [P, k, h]), partition-major biases, ones
        wv_sb = consts.tile([P, kv, h], CT)
        nc.scalar.dma_start(_w(wv_sb[:, 0, :]), _w(wvT[ds(0, P), :]))
        nc.scalar.dma_start(
            _w(wv_sb[:, ds(1, kv - 1), :]),
            _w(wvT[ds(P, (kv - 1) * P), :].rearrange("(k p) h -> p k h", p=P)),
        )
        wt_sb = consts.tile([P, kt, h], CT)
        nc.scalar.dma_start(_w(wt_sb[:]), _w(wtT.rearrange("(k p) h -> p k h", p=P)))
        bv_sb = consts.tile([P, mh], F32)
        nc.scalar.dma_start(bv_sb[:], bvp[:, :])
        bt_sb = consts.tile([P, mh], F32)
        nc.scalar.dma_start(bt_sb[:], btp[:, :])
        ones_sb = consts.tile([P, P], CT)
        nc.scalar.dma_start(_w(ones_sb[:]), _w(onesd[:, :]))
        ones_col = ones_sb[:, 0:1]
        ones_row = ones_sb[0:1, :]

        # PE warm-up: ~20 dummy matmuls on a memset tile while the first input
        # DMAs are still in flight, so HAM is at full clock for real work.
        warm_sb = consts.tile([P, fd], CT)
        nc.vector.memset(warm_sb[:], 0.0)
        warm_ps = ps_repl.tile([P, fd], F32, tag="ps_repl")
        for _ in range(20):
            nc.tensor.matmul(warm_ps[:], _w(warm_sb[:, 0:P]), _w(warm_sb[:]))
        nc.scalar.activation(_w(warm_sb[:, 0:P]), warm_ps[:, 0:P], AF.Copy)

        def proj(m_range, kk, w_sb, b_sb, x_sb, y_sb, ysq_sb):
            """y[:, m, :] = W[:, :, m-slice].T @ x + b ; ysq = y*y"""
            for m in m_range:
                for n2 in range(nsm):
                    pv = ps_mm.tile([P, fdm], F32, tag="ps_mm")
                    for k in range(kk):
                        nc.tensor.matmul(
                            pv[:],
                            _w(w_sb[:, k, ts(m, P)]),
                            _w(x_sb[:, k, ds(n2 * fdm, fdm)]),
                            start=(k == 0),
                            stop=(k == kk - 1),
                        )
                    nc.scalar.activation(
                        _w(y_sb[:, m, ds(n2 * fdm, fdm)]), pv[:], AF.Identity,
                        bias=b_sb[:, ds(m, 1)],
                    )
                    nc.vector.tensor_mul(
                        _w(ysq_sb[:, m, ds(n2 * fdm, fdm)]),
                        y_sb[:, m, ds(n2 * fdm, fdm)],
                        y_sb[:, m, ds(n2 * fdm, fdm)],
                    )

        def norm_rows(ysq_sb, tag):
            """Per-column sqrt(sum_h ysq) as ns rows of [1, fd] (dtype CT)."""
            rows = []
            for n2 in range(ns):
                pn = ps_norm.tile([1, fd], F32, tag="ps_norm")
                for m in range(mh):
                    nc.tensor.matmul(
                        pn[:],
                        _w(ones_col),
                        _w(ysq_sb[:, m, ds(n2 * fd, fd)]),
                        start=(m == 0),
                        stop=(m == mh - 1),
                    )
                nrow = row_pool.tile([1, fd], CT, tag=f"n_{tag}{n2}")
                nc.scalar.activation(_w(nrow[:]), pn[:], AF.Sqrt)
                rows.append(nrow)
            return rows

        def repl_recip(rows, dest_sb):
            """Broadcast 1/row across 128 partitions into dest_sb [P, s] f32."""
            for n2 in range(ns):
                rp = ps_repl.tile([P, fd], F32, tag="ps_repl")
                nc.tensor.matmul(rp[:], _w(ones_row), _w(rows[n2][:]))
                nc.vector.reciprocal_approx_fast(
                    out=dest_sb[:, ds(n2 * fd, fd)], in_=rp[:]
                )

        for b in range(bpc):
            # --- input loads (per-k chunks; Tile orders/overlaps them)
            vis_sb = vis_pool.tile([P, kv, s], CT)
            for k in range(kv):
                nc.sync.dma_start(_w(vis_sb[:, k, :]), _w(visT[b, ds(k * P, P), :]))
            txt_sb = txt_pool.tile([P, kt, s], CT)
            for k in range(kt):
                nc.sync.dma_start(_w(txt_sb[:, k, :]), _w(txtT[b, ds(k * P, P), :]))

            vt_sb = vt_pool.tile([P, mh, s], CT)
            tt_sb = tt_pool.tile([P, mh, s], CT)
            vsq_sb = sq_pool.tile([P, mh, s], CT, tag="vsq")
            tsq_sb = sq_pool.tile([P, mh, s], CT, tag="tsq")

            # --- projections + v-norm chain interleaved to keep PE dense
            proj(range(mh), kv, wv_sb, bv_sb, vis_sb, vt_sb, vsq_sb)
            proj(range(0, mh // 2), kt, wt_sb, bt_sb, txt_sb, tt_sb, tsq_sb)
            rvn_rows = norm_rows(vsq_sb, "v")
            # replicate 1/vn and fold into vT; the chain overlaps proj-t m23
            rvn_bc = rvn_pool.tile([P, s], F32)
            repl_recip(rvn_rows, rvn_bc)
            proj(range(mh // 2, mh), kt, wt_sb, bt_sb, txt_sb, tt_sb, tsq_sb)
            for n2 in range(ns):
                for m in range(mh):
                    nc.vector.tensor_mul(
                        _w(vt_sb[:, m, ds(n2 * fd, fd)]),
                        vt_sb[:, m, ds(n2 * fd, fd)],
                        rvn_bc[:, ds(n2 * fd, fd)],
                    )  # [P, fd] chunks: first chunks unblock dots i=0 sooner

            rtn_rows = norm_rows(tsq_sb, "t")
            rtn_bc = rtn_pool.tile([P, s], F32)
            repl_recip(rtn_rows, rtn_bc)

            # --- dots + epilogue
            out_sb = None
            for i in range(ms):
                pds = []
                for jc in range(nsm):
                    pd = ps_mm.tile([P, fdm], F32, tag="ps_mm")
                    for hc in range(mh):
                        nc.tensor.matmul(
                            pd[:],
                            _w(vt_sb[:, hc, ts(i, P)]),
                            _w(tt_sb[:, hc, ds(jc * fdm, fdm)]),
                            start=(hc == 0),
                            stop=(hc == mh - 1),
                        )
                    pds.append(pd)
                out_sb = out_pool.tile([P, s], F32)
                for jc in range(nsm):
                    nc.vector.tensor_mul(
                        out_sb[:, ds(jc * fdm, fdm)],
                        pds[jc][:],
                        rtn_bc[:, ds(jc * fdm, fdm)],
                    )
                nc.gpsimd.dma_start(out[b, ds(i * P, P), :], out_sb[:])

    nc.compile()
    return nc


_ONES = np.ones((P, P), dtype=np.float32)

_CACHE = {}


def _get_nc(dtype="bf16"):
    if dtype not in _CACHE:
        _CACHE[dtype] = build(dtype=dtype)
    return _CACHE[dtype]


def _prep_in_maps(visual_features, text_features, Wv, bv, Wt, bt, dtype="bf16"):
    import ml_dtypes

    f = np.float32
    ct = ml_dtypes.bfloat16 if dtype == "bf16" else f
    wvT = np.ascontiguousarray(np.asarray(Wv, dtype=f).T).astype(ct)  # [VD, H]
    wtT = np.ascontiguousarray(np.asarray(Wt, dtype=f).T).astype(ct)  # [TD, H]
    bvp = np.ascontiguousarray(np.asarray(bv, dtype=f).reshape(H // P, P).T)
    btp = np.ascontiguousarray(np.asarray(bt, dtype=f).reshape(H // P, P).T)
    ones = _ONES.astype(ct)
    vis = np.asarray(visual_features, dtype=f)
    txt = np.asarray(text_features, dtype=f)
    in_maps = []
    for c in range(NCORES):
        sl = slice(c * BPC, (c + 1) * BPC)
        in_maps.append({
            "visT": np.ascontiguousarray(vis[sl].transpose(0, 2, 1)).astype(ct),
            "txtT": np.ascontiguousarray(txt[sl].transpose(0, 2, 1)).astype(ct),
            "wvT": wvT,
            "wtT": wtT,
            "bvp": bvp,
            "btp": btp,
            "ones": ones,
        })
    return in_maps


def run(inputs, trace=False, tmpdir=None, dtype="bf16"):
    """Returns (full_output, BassKernelResults)."""
    from concourse.bass_utils import run_bass_kernel_spmd

    nc = _get_nc(dtype)
    in_maps = _prep_in_maps(**inputs, dtype=dtype)
    res = run_bass_kernel_spmd(
        nc, in_maps, core_ids=list(range(NCORES)), trace=trace, tmpdir=tmpdir
    )
    outp = np.concatenate([res.results[c]["out"] for c in range(NCORES)], axis=0)
    return outp, res


def kernel(**inputs) -> np.ndarray:
    outp, _ = run(inputs, trace=False)
    return outp
